# revision 10
# baseline (speedup 1.0000x reference)
"""CameraMemory circle-loss kernel for 8 Trainium2 NeuronCores.

Strategy
--------
reference computes:
    x        = normalize(inputs)                      [B, D]
    out      = (x @ features.T + 1) / 2               [B, N]
    loss_p   = sum over {pids[j]==targets[b]}  of exp(20*(1-out)^2)   (alpha_p relu'd)
    loss_n   = sum over {pids[j]!=targets[b] and camids[j]==cams[b]}
                                               of exp(20*out^2)
    return log1p(loss_p * loss_n)

With s = x.f (raw cosine), 20*out^2 = 5*(1+s)^2 and 20*(1-out)^2 = 5*(1-s)^2.

- loss_n's camera mask is made *block diagonal* by sorting the memory bank by
  camid on the host and grouping batch rows by cam.  Each device then only
  computes similarity blocks [rows(cam) x cols(cam)] -- no masks needed on
  device, 1/8th the matmul work, while still streaming the full feature shard
  from HBM once (the memory roofline this problem targets).
- Device sums exp(5*(1+s)^2) over every cam-matching pair.  The tiny subset
  that must be *excluded* (pid also matches: ~3k pairs of 25.6M) is computed
  on host and subtracted.
- loss_p only involves pairs with pids[j]==targets[b] (~26k of 25.6M); that
  sparse sum is done on host directly.

Device kernel (identical program on all 8 cores, different data):
  for each of CPC chunks:  (chunk = one cam row-group x 500 bank columns)
    load fT tile  [2 x 128 x 500]  (features.T slice, f32)
    load lhsT     [2 x 128 x 64]   (normalized-x columns for this cam's rows)
    psum  = lhsT.T @ fT            (PE, K=256 via 2 accumulating matmuls)
    sq    = Square(sqrt5*s + sqrt5)         -> 5*(1+s)^2        (ACT)
    ex    = Exp(sq), accum_out -> partial sums per row slot     (ACT)
  single DMA of all [64 x CPC] partials back to HBM.

Column padding inside a chunk contributes exp(5) per padded column to every
row slot (s=0); the host subtracts npad*e5 exactly.
"""

import os

import numpy as np

B, D = 256, 256
TEMP = 0.05
EPS = 1e-12
NCORES = 8
W = 500      # columns per chunk (<=512 PSUM/matmul limit)
NSLOT = 64   # row slots per chunk
KT = 2       # contraction tiles (D = KT * 128)
SQRT5 = float(np.sqrt(5.0))

# exp value a zero-padded column contributes on device: Exp(Square(bias))
_B32 = np.float32(SQRT5)
E5_PAD = float(np.exp(np.float64(np.float32(_B32 * _B32))))

LAST_EXEC_TIME_NS = None
LAST_TRACE_PATH = None

_NC_CACHE = {}


def _build_bass(cpc, use_f32r):
    import concourse.bacc as bacc
    import concourse.mybir as mybir
    import concourse.tile as tile

    dt = mybir.dt
    mdt = dt.float32r if use_f32r else dt.float32
    AF = mybir.ActivationFunctionType

    CW = W + NSLOT  # combined tile: [0:W] features.T cols, [W:] lhsT cols
    nc = bacc.Bacc("TRN2", target_bir_lowering=False)
    fl = nc.dram_tensor("fl", [cpc, KT, 128, CW], mdt, kind="ExternalInput")
    out = nc.dram_tensor("out", [NSLOT, cpc], dt.float32, kind="ExternalOutput")

    with tile.TileContext(nc) as tc:
        with (
            tc.tile_pool(name="fpool", bufs=4) as fpool,
            tc.tile_pool(name="psum", bufs=4, space="PSUM") as pspool,
            tc.tile_pool(name="work", bufs=3) as wpool,
            tc.tile_pool(name="res", bufs=1) as rpool,
        ):
            allparts = rpool.tile([NSLOT, cpc], dt.float32)
            bias_t = rpool.tile([128, 1], dt.float32)
            nc.vector.memset(bias_t, SQRT5)
            for c in range(cpc):
                t0 = fpool.tile([128, CW], mdt, tag="t0")
                t1 = fpool.tile([128, CW], mdt, tag="t1")
                nc.sync.dma_start(out=t0, in_=fl[c, 0])
                nc.sync.dma_start(out=t1, in_=fl[c, 1])
                ps = pspool.tile([NSLOT, W], dt.float32)
                nc.tensor.matmul(
                    ps, lhsT=t0[:, W:CW], rhs=t0[:, 0:W], start=True, stop=False
                )
                nc.tensor.matmul(
                    ps, lhsT=t1[:, W:CW], rhs=t1[:, 0:W], start=False, stop=True
                )
                sq = wpool.tile([NSLOT, W], dt.float32, tag="sq")
                nc.scalar.activation(
                    sq, ps, AF.Square, bias=bias_t[:NSLOT, :], scale=SQRT5
                )
                ex = wpool.tile([NSLOT, W], dt.float32, tag="ex")
                nc.scalar.activation(ex, sq, AF.Exp, accum_out=allparts[:, c : c + 1])
            nc.sync.dma_start(out=out[:, :], in_=allparts)
    nc.compile()
    return nc


def _host_sparse_sums(x, features, targets, cams, pids, camids):
    """loss_p (all pid-matching pairs) and J (pid AND cam matching pairs),
    mirroring the reference formulas, summed in float64."""
    loss_p = 0.0
    jsum = 0.0
    order_p = np.argsort(pids, kind="stable")
    pids_sorted = pids[order_p]
    for t in np.unique(targets):
        rows = np.flatnonzero(targets == t)
        lo = np.searchsorted(pids_sorted, t, "left")
        hi = np.searchsorted(pids_sorted, t, "right")
        js = order_p[lo:hi]
        if len(js) == 0 or len(rows) == 0:
            continue
        sub = x[rows] @ features[js].T                      # [r, m] f32
        o = ((sub + np.float32(1.0)) * np.float32(0.5)).astype(np.float32)
        ap = np.maximum(np.float32(1.0) - o, np.float32(0.0))
        termp = np.exp(-ap * (o - np.float32(1.0)) / np.float32(TEMP))
        loss_p += termp.sum(dtype=np.float64)
        cam_eq = camids[js][None, :] == cams[rows][:, None]
        if cam_eq.any():
            an = np.maximum(o, np.float32(0.0))
            termn = np.exp(an * o / np.float32(TEMP))
            jsum += termn[cam_eq].sum(dtype=np.float64)
    return loss_p, jsum


def _prepare(inputs):
    """Host-side prep: normalize, sparse sums, sort/pack device data, build+compile
    the bass module.  Returns a dict with everything kernel() needs to run+reduce."""
    x_in = np.ascontiguousarray(np.asarray(inputs["inputs"], dtype=np.float32))
    features = np.ascontiguousarray(np.asarray(inputs["features"], dtype=np.float32))
    targets = np.asarray(inputs["targets"]).astype(np.int64)
    cams = np.asarray(inputs["cams"]).astype(np.int64)
    pids = np.asarray(inputs["pids"]).astype(np.int64)
    camids = np.asarray(inputs["camids"]).astype(np.int64)
    n_bank = features.shape[0]

    # F.normalize(inputs, dim=1) in f32, as the reference does
    nrm = np.sqrt(np.sum(x_in * x_in, axis=1, keepdims=True, dtype=np.float32))
    x = x_in / np.maximum(nrm, np.float32(EPS))

    # -------- host-side sparse branches --------
    loss_p, jsum = _host_sparse_sums(x, features, targets, cams, pids, camids)

    # -------- device-side dense cam-blocked branch --------
    perm = np.argsort(camids, kind="stable")
    camids_s = camids[perm]
    fT_s = np.ascontiguousarray(features[perm].T)           # [D, N] f32

    chunks = []  # (rows_group, col_start, col_end) in sorted-bank coords
    for c in np.unique(cams):
        rows_c = np.flatnonzero(cams == c)
        jlo = np.searchsorted(camids_s, c, "left")
        jhi = np.searchsorted(camids_s, c, "right")
        if jhi == jlo or len(rows_c) == 0:
            continue
        for g in range(0, len(rows_c), NSLOT):
            rows_g = rows_c[g : g + NSLOT]
            for s0 in range(jlo, jhi, W):
                chunks.append((rows_g, s0, min(s0 + W, jhi)))

    cpc = max(1, (len(chunks) + NCORES - 1) // NCORES)
    CW = W + NSLOT
    fl_arr = np.zeros((NCORES, cpc, KT, 128, CW), dtype=np.float32)
    meta = [[None] * cpc for _ in range(NCORES)]
    xT = np.ascontiguousarray(x.T)                           # [D, B]
    for idx, (rows_g, s0, s1) in enumerate(chunks):
        m, q = idx // cpc, idx % cpc
        w = s1 - s0
        for k in range(KT):
            fl_arr[m, q, k, :, :w] = fT_s[k * 128 : (k + 1) * 128, s0:s1]
            fl_arr[m, q, k, :, W : W + len(rows_g)] = xT[
                k * 128 : (k + 1) * 128, rows_g
            ]
        meta[m][q] = (rows_g, w)

    use_f32r = os.environ.get("KERNEL_F32R", "0") == "1"
    key = (cpc, use_f32r)
    if key not in _NC_CACHE:
        _NC_CACHE[key] = _build_bass(cpc, use_f32r)
    nc = _NC_CACHE[key]

    return {
        "nc": nc,
        "cpc": cpc,
        "in_maps": [{"fl": fl_arr[m]} for m in range(NCORES)],
        "meta": meta,
        "loss_p": loss_p,
        "jsum": jsum,
    }


def _reduce(prep, results):
    """Combine per-core device partials with the host-side sparse sums."""
    meta, cpc = prep["meta"], prep["cpc"]
    loss_n_dev = 0.0
    for m in range(NCORES):
        o = results[m]["out"]                                # [NSLOT, cpc]
        for q in range(cpc):
            if meta[m][q] is None:
                continue
            rows_g, w = meta[m][q]
            vals = o[: len(rows_g), q].astype(np.float64) - (W - w) * E5_PAD
            loss_n_dev += vals.sum()

    loss_n = loss_n_dev - prep["jsum"]
    lp = np.float64(np.float32(prep["loss_p"]))
    ln = np.float64(np.float32(loss_n))
    return np.float32(np.log1p(lp * ln))


def kernel(**inputs):
    prep = _prepare(inputs)
    from concourse.bass_utils import run_bass_kernel_spmd

    res = run_bass_kernel_spmd(
        prep["nc"], prep["in_maps"], core_ids=list(range(NCORES))
    )
    return _reduce(prep, res.results)


# revision 42
# speedup vs baseline: 381.3961x; 381.3961x over previous
"""CameraMemory circle-loss kernel for 8 Trainium2 NeuronCores.

Strategy
--------
reference computes:
    x        = normalize(inputs)                      [B, D]
    out      = (x @ features.T + 1) / 2               [B, N]
    loss_p   = sum over {pids[j]==targets[b]}  of exp(20*(1-out)^2)   (alpha_p relu'd)
    loss_n   = sum over {pids[j]!=targets[b] and camids[j]==cams[b]}
                                               of exp(20*out^2)
    return log1p(loss_p * loss_n)

With s = x.f (raw cosine), 20*out^2 = 5*(1+s)^2 and 20*(1-out)^2 = 5*(1-s)^2.

- loss_n's camera mask is made *block diagonal* by sorting the memory bank by
  camid on the host and grouping batch rows by cam.  Each device then only
  computes similarity blocks [rows(cam) x cols(cam)] -- no masks needed on
  device, 1/8th the matmul work, while still streaming the full feature shard
  from HBM once (the memory roofline this problem targets).
- Device sums exp(5*(1+s)^2) over every cam-matching pair.  The tiny subset
  that must be *excluded* (pid also matches: ~3k pairs of 25.6M) is computed
  on host and subtracted.
- loss_p only involves pairs with pids[j]==targets[b] (~26k of 25.6M); that
  sparse sum is done on host directly.

Device kernel (identical program on all 8 cores, different data):
  for each of CPC chunks:  (chunk = one cam row-group x 500 bank columns)
    load fT tile  [2 x 128 x 500]  (features.T slice, f32)
    load lhsT     [2 x 128 x 64]   (normalized-x columns for this cam's rows)
    psum  = lhsT.T @ fT            (PE, K=256 via 2 accumulating matmuls)
    sq    = Square(sqrt5*s + sqrt5)         -> 5*(1+s)^2        (ACT)
    ex    = Exp(sq), accum_out -> partial sums per row slot     (ACT)
  single DMA of all [64 x CPC] partials back to HBM.

Column padding inside a chunk contributes exp(5) per padded column to every
row slot (s=0); the host subtracts npad*e5 exactly.
"""

import os

import numpy as np

B, D = 256, 256
TEMP = 0.05
EPS = 1e-12
NCORES = 8
W = 512      # columns per chunk (= max matmul N; exactly one 2KB PSUM bank)
NSLOT = 64   # row slots per chunk
KT = 2       # contraction tiles (D = KT * 128)
G4 = 4       # chunks fused per PSUM tile / ACT pass
DEFAULT_DTYPE = "bf16"
SQRT5 = float(np.sqrt(5.0))

# a zero-padded column (s=0) contributes exp(5*(0+1)^2) = e^5 on device
E5 = float(np.exp(np.float64(5.0)))

LAST_EXEC_TIME_NS = None
LAST_TRACE_PATH = None

_NC_CACHE = {}


def _dve_super(s, spc):
    """Which supers compute the square on DVE (e^-5-shifted exps).
    Must match between device build and host reduction.  Alternating ACT/DVE
    supers pipelines best: ACT runs super s's exps while DVE squares s+1."""
    return s % 2 == 1


def _mdt(dtype_name):
    import concourse.mybir as mybir

    return {
        "f32": mybir.dt.float32,
        "f32r": mybir.dt.float32r,
        "bf16": mybir.dt.bfloat16,
    }[dtype_name]


def _build_bass(cpc, dtype_name):
    import concourse.bacc as bacc
    import concourse.mybir as mybir
    import concourse.tile as tile

    dt = mybir.dt
    mdt = _mdt(dtype_name)
    AF = mybir.ActivationFunctionType

    CW = W + NSLOT  # per chunk: [0:W] features.T cols, [W:] lhsT cols
    assert cpc % G4 == 0
    spc = cpc // G4
    nc = bacc.Bacc("TRN2", target_bir_lowering=False)
    fl = nc.dram_tensor("fl", [spc, KT, 128, G4 * CW], mdt, kind="ExternalInput")
    out = nc.dram_tensor("out", [NSLOT, spc], dt.float32, kind="ExternalOutput")

    with tile.TileContext(nc) as tc:
        with (
            tc.tile_pool(name="fpool", bufs=4) as fpool,
            tc.tile_pool(name="psum", bufs=2, space="PSUM") as pspool,
            tc.tile_pool(name="work", bufs=4) as wpool,
            tc.tile_pool(name="res", bufs=1) as rpool,
        ):
            allparts = rpool.tile([NSLOT, spc], dt.float32)
            bias_t = rpool.tile([128, 1], dt.float32)
            nc.vector.memset(bias_t, SQRT5)
            # prime the ACT exp table set before the steady-state loop
            scratch = rpool.tile([128, 1], dt.float32)
            nc.vector.memset(scratch, 0.0)
            scratch2 = rpool.tile([128, 1], dt.float32)
            nc.scalar.activation(scratch2, scratch, AF.Exp)
            for s in range(spc):
                # one tile per (ktile, chunk-pair) so matmuls start after 1/2
                # of a super's data instead of all of it
                tiles = {}
                for k in range(KT):
                    for h in range(2):
                        t = fpool.tile([128, 2 * CW], mdt, tag=f"t{k}{h}")
                        nc.sync.dma_start(
                            out=t, in_=fl[s, k][:, h * 2 * CW : (h + 1) * 2 * CW]
                        )
                        tiles[k, h] = t
                ps = pspool.tile([NSLOT, G4 * W], dt.float32)
                for i in range(G4):
                    h, j = i // 2, i % 2
                    for k in range(KT):
                        t = tiles[k, h]
                        nc.tensor.matmul(
                            ps[:, i * W : (i + 1) * W],
                            lhsT=t[:, j * CW + W : (j + 1) * CW],
                            rhs=t[:, j * CW : j * CW + W],
                            start=(k == 0),
                            stop=(k == KT - 1),
                        )
                # square: split between ACT (Square func) and DVE (two-pass
                # (s+2)*s, whose exp is e^-5-shifted; host rescales by e^5)
                on_dve = _dve_super(s, spc)
                sq = wpool.tile([NSLOT, G4 * W], dt.float32, tag="sq")
                if on_dve:
                    v = wpool.tile([NSLOT, G4 * W], dt.float32, tag="v")
                    nc.vector.tensor_scalar(
                        v, ps, 2.0, None, op0=mybir.AluOpType.add
                    )
                    nc.vector.tensor_tensor(
                        out=sq, in0=v, in1=ps, op=mybir.AluOpType.mult
                    )
                    exp_scale = 5.0
                else:
                    nc.scalar.activation(
                        sq, ps, AF.Square, bias=bias_t[:NSLOT, :], scale=SQRT5
                    )
                    exp_scale = 1.0
                ex = wpool.tile([NSLOT, G4 * W], dt.float32, tag="ex")
                nc.scalar.activation(
                    ex, sq, AF.Exp, scale=exp_scale,
                    accum_out=allparts[:, s : s + 1],
                )
            nc.sync.dma_start(out=out[:, :], in_=allparts)
    nc.compile()
    return nc


def _host_sparse_sums(x, features, targets, cams, pids, camids):
    """loss_p (all pid-matching pairs) and J (pid AND cam matching pairs),
    mirroring the reference formulas, summed in float64."""
    loss_p = 0.0
    jsum = 0.0
    order_p = np.argsort(pids, kind="stable")
    pids_sorted = pids[order_p]
    for t in np.unique(targets):
        rows = np.flatnonzero(targets == t)
        lo = np.searchsorted(pids_sorted, t, "left")
        hi = np.searchsorted(pids_sorted, t, "right")
        js = order_p[lo:hi]
        if len(js) == 0 or len(rows) == 0:
            continue
        sub = x[rows] @ features[js].T                      # [r, m] f32
        o = ((sub + np.float32(1.0)) * np.float32(0.5)).astype(np.float32)
        ap = np.maximum(np.float32(1.0) - o, np.float32(0.0))
        termp = np.exp(-ap * (o - np.float32(1.0)) / np.float32(TEMP))
        loss_p += termp.sum(dtype=np.float64)
        cam_eq = camids[js][None, :] == cams[rows][:, None]
        if cam_eq.any():
            an = np.maximum(o, np.float32(0.0))
            termn = np.exp(an * o / np.float32(TEMP))
            jsum += termn[cam_eq].sum(dtype=np.float64)
    return loss_p, jsum


def _prepare(inputs):
    """Host-side prep: normalize, sparse sums, sort/pack device data, build+compile
    the bass module.  Returns a dict with everything kernel() needs to run+reduce."""
    x_in = np.ascontiguousarray(np.asarray(inputs["inputs"], dtype=np.float32))
    features = np.ascontiguousarray(np.asarray(inputs["features"], dtype=np.float32))
    targets = np.asarray(inputs["targets"]).astype(np.int64)
    cams = np.asarray(inputs["cams"]).astype(np.int64)
    pids = np.asarray(inputs["pids"]).astype(np.int64)
    camids = np.asarray(inputs["camids"]).astype(np.int64)
    n_bank = features.shape[0]

    # F.normalize(inputs, dim=1) in f32, as the reference does
    nrm = np.sqrt(np.sum(x_in * x_in, axis=1, keepdims=True, dtype=np.float32))
    x = x_in / np.maximum(nrm, np.float32(EPS))

    # -------- host-side sparse branches --------
    loss_p, jsum = _host_sparse_sums(x, features, targets, cams, pids, camids)

    # -------- device-side dense cam-blocked branch --------
    perm = np.argsort(camids, kind="stable")
    camids_s = camids[perm]
    fT_s = np.ascontiguousarray(features[perm].T)           # [D, N] f32

    chunks = []  # (rows_group, col_start, col_end) in sorted-bank coords
    for c in np.unique(cams):
        rows_c = np.flatnonzero(cams == c)
        jlo = np.searchsorted(camids_s, c, "left")
        jhi = np.searchsorted(camids_s, c, "right")
        if jhi == jlo or len(rows_c) == 0:
            continue
        for g in range(0, len(rows_c), NSLOT):
            rows_g = rows_c[g : g + NSLOT]
            n0 = len(chunks)
            for s0 in range(jlo, jhi, W):
                chunks.append((rows_g, s0, min(s0 + W, jhi)))
            while (len(chunks) - n0) % G4 != 0:  # keep supers group-pure
                chunks.append((rows_g, jhi, jhi))

    cpc = max(1, (len(chunks) + NCORES - 1) // NCORES)
    cpc = ((cpc + G4 - 1) // G4) * G4                         # multiple of G4
    CW = W + NSLOT
    dtype_name = os.environ.get("KERNEL_DTYPE", DEFAULT_DTYPE)
    if dtype_name == "bf16":
        import ml_dtypes

        np_dt = ml_dtypes.bfloat16
    else:
        np_dt = np.float32
    fl_arr = np.zeros((NCORES, cpc, KT, 128, CW), dtype=np_dt)
    meta = [[None] * cpc for _ in range(NCORES)]
    xT = np.ascontiguousarray(x.T)                           # [D, B]
    for idx, (rows_g, s0, s1) in enumerate(chunks):
        m, q = idx // cpc, idx % cpc
        w = s1 - s0
        for k in range(KT):
            fl_arr[m, q, k, :, :w] = fT_s[k * 128 : (k + 1) * 128, s0:s1].astype(np_dt)
            fl_arr[m, q, k, :, W : W + len(rows_g)] = xT[
                k * 128 : (k + 1) * 128, rows_g
            ].astype(np_dt)
        meta[m][q] = (rows_g, w)
    # device layout: [spc, KT, 128, G4*CW] — interleave the G4 chunks per super
    fl_dev = np.ascontiguousarray(
        fl_arr.reshape(NCORES, cpc // G4, G4, KT, 128, CW)
        .transpose(0, 1, 3, 4, 2, 5)
        .reshape(NCORES, cpc // G4, KT, 128, G4 * CW)
    )

    key = (cpc, dtype_name)
    if key not in _NC_CACHE:
        _NC_CACHE[key] = _build_bass(cpc, dtype_name)
    nc = _NC_CACHE[key]

    return {
        "nc": nc,
        "cpc": cpc,
        "in_maps": [{"fl": fl_dev[m]} for m in range(NCORES)],
        "meta": meta,
        "loss_p": loss_p,
        "jsum": jsum,
    }


def _reduce(prep, results):
    """Combine per-core device partials with the host-side sparse sums."""
    meta, cpc = prep["meta"], prep["cpc"]
    loss_n_dev = 0.0
    for m in range(NCORES):
        o = results[m]["out"]                                # [NSLOT, spc]
        for s in range(cpc // G4):
            ms = [meta[m][s * G4 + i] for i in range(G4)]
            if all(mm is None for mm in ms):
                continue
            assert all(mm is not None for mm in ms), "super straddles dead pad"
            rows_g = ms[0][0]
            assert all(
                mm[0] is rows_g or np.array_equal(mm[0], rows_g) for mm in ms
            )
            npad = G4 * W - sum(mm[1] for mm in ms)
            part = o[: len(rows_g), s].astype(np.float64)
            if _dve_super(s, cpc // G4):       # DVE square: e^-5-scaled terms
                vals = E5 * (part - npad)
            else:
                vals = part - npad * E5
            loss_n_dev += vals.sum()

    loss_n = loss_n_dev - prep["jsum"]
    lp = np.float64(np.float32(prep["loss_p"]))
    ln = np.float64(np.float32(loss_n))
    return np.float32(np.log1p(lp * ln))


def kernel(**inputs):
    prep = _prepare(inputs)
    from concourse.bass_utils import run_bass_kernel_spmd

    res = run_bass_kernel_spmd(
        prep["nc"], prep["in_maps"], core_ids=list(range(NCORES))
    )
    return _reduce(prep, res.results)
